# revision 3
# baseline (speedup 1.0000x reference)
"""Dense associative-embedding loss on 8 Trainium2 NeuronCores.

Math (reference):
    g[b, n, p, c] = pred[b, c, inds[b, n, p]]
    centers       = mean_p(g)                              # [B, N, C]
    pull          = 0.25 * sum_{b,n} sum_c (mean_p g^2 - centers^2)
    s[b, n]       = sum_c centers
    push          = 0.25 * sum_b sum_{i != j} relu(2 - |s_i - s_j|) / (N(N-1))

Only B*N*P*C = 262144 of pred's 33.5M elements are ever read, so the kernel
is a sparse gather. The host re-lays pred channel-last ([b, hw, c] flat), so
each point's 8 channels are one contiguous 32-byte run. On-chip, indirect
DMAs gather 128 points per instruction (the HW contract is one descriptor
per SBUF partition, descriptor length = dest row size): 32 instructions
fill g[128, 256] with point slot (p, k) at partition p = b*64 + n*2 + pp//32,
col k = pp % 32 (2 partitions per instance).

The gathers serialize on the single Pool dynamic queue (~1.4us each) and are
the critical path; everything else hides under them. Reduction: per-partition
strided X-reduces give R1 = sum_k g, R2 = sum_k g^2 per (partition, channel);
the tiny [128, 16] result goes to the host, which contracts the two
partitions of each instance and applies the pull/push normalization (the
unshard step).
"""

import numpy as np

_B, _C, _H, _W = 16, 8, 512, 512
_HW = _H * _W
_N, _P = 32, 64
_NCORES = 8
_BP = _B // _NCORES              # batch elements per core
_NI = _BP * _N                   # instances per core = 64
_KCOLS = 32                      # point slots per partition
_NGATHER = _P // _KCOLS          # partitions per instance = 2
_V = _BP * _HW * _C              # flat pred elements per core (channel-last)

_MARGIN = 2.0
_PULL_W = 0.25
_PUSH_W = 0.25

_program = None


def _build_program():
    import concourse.bacc as bacc
    import concourse.bass as bass
    import concourse.mybir as mybir
    import concourse.tile as tile

    f32 = mybir.dt.float32
    i32 = mybir.dt.int32
    X = mybir.AxisListType.X

    nc = bacc.Bacc("TRN2", target_bir_lowering=False, debug=False)

    pred_d = nc.dram_tensor("pred", [_V, 1], f32, kind="ExternalInput")
    idx_d = nc.dram_tensor("idx", [128, _KCOLS], i32, kind="ExternalInput")
    out_d = nc.dram_tensor("out", [128, 2 * _C], f32, kind="ExternalOutput")

    with tile.TileContext(nc) as tc:
        with tc.tile_pool(name="sb", bufs=1) as sb:
            idx_t = sb.tile([128, _KCOLS], i32)
            nc.sync.dma_start(out=idx_t[:], in_=idx_d[:])

            # Gather in chunks; each chunk's square runs under the shadow of
            # the remaining gathers. The final strided reduces cover all 32
            # point columns in one pass each.
            NCHUNK = 4
            KC = _KCOLS // NCHUNK
            g = sb.tile([128, _KCOLS * _C], f32)
            g2 = sb.tile([128, _KCOLS * _C], f32)
            for q in range(NCHUNK):
                for kk in range(KC):
                    k = q * KC + kk
                    nc.gpsimd.indirect_dma_start(
                        out=g[:, k * _C : (k + 1) * _C],
                        out_offset=None,
                        in_=pred_d[:, :],
                        in_offset=bass.IndirectOffsetOnAxis(
                            ap=idx_t[:, k : k + 1], axis=0
                        ),
                    )
                sl = slice(q * KC * _C, (q + 1) * KC * _C)
                nc.vector.tensor_mul(g2[:, sl], g[:, sl], g[:, sl])
            R = sb.tile([128, 2 * _C], f32)
            nc.vector.reduce_sum(
                out=R[:, 0:_C],
                in_=g[:].rearrange("p (k c) -> p c k", c=_C),
                axis=X,
            )
            nc.vector.reduce_sum(
                out=R[:, _C:],
                in_=g2[:].rearrange("p (k c) -> p c k", c=_C),
                axis=X,
            )
            nc.sync.dma_start(out=out_d[:], in_=R[:])

    nc.finalize()
    return nc


def _get_program():
    global _program
    if _program is None:
        _program = _build_program()
    return _program


def _make_in_maps(pred, inds):
    pred = np.asarray(pred)
    inds = np.asarray(inds).astype(np.int64)
    in_maps = []
    for mcore in range(_NCORES):
        psh = pred[_BP * mcore : _BP * (mcore + 1)]   # [BP, C, H, W]
        ish = inds[_BP * mcore : _BP * (mcore + 1)]   # [BP, N, P]
        # channel-last flat layout: element (b, hw, c) at ((b*HW + hw)*C + c)
        pcl = np.ascontiguousarray(
            psh.reshape(_BP, _C, _HW).transpose(0, 2, 1), dtype=np.float32
        ).reshape(_V, 1)
        # idx[p, k]: partition p = b*64 + n*2 + pp//32, col k = pp % 32
        # element offset of point (b, n, pp) = (b*HW + inds[b,n,pp]) * C
        off = (ish + (np.arange(_BP, dtype=np.int64) * _HW)[:, None, None]) * _C
        off = off.reshape(_BP, _N, _NGATHER, _KCOLS)       # pp = half*32 + k
        idx = off.transpose(0, 1, 2, 3).reshape(_BP * _N * _NGATHER, _KCOLS)
        in_maps.append(
            {
                "pred": pcl,
                "idx": np.ascontiguousarray(idx, dtype=np.int32),
            }
        )
    return in_maps


def _combine(core_outs):
    """core_outs: per-core [128, 16] = R1 | R2 per (partition, channel).
    Partition p = instance m = p//2 (2 partitions per instance)."""
    pull = 0.0
    push = 0.0
    for R in core_outs:
        R = np.asarray(R, dtype=np.float64)
        R1 = R[:, 0:_C].reshape(_NI, _NGATHER, _C).sum(axis=1)  # [64, 8]
        R2 = R[:, _C:].reshape(_NI, _NGATHER, _C).sum(axis=1)   # [64, 8]
        # pull_inst = mean_p |g|^2 - |center|^2, summed over channels
        pull += (R2.sum(1) / _P - (R1 * R1).sum(1) / (_P * _P)).sum()
        # push: per image, pairwise on s = sum_c centers
        s = R1.sum(1) / _P                                      # [64]
        for b in range(_BP):
            sb = s[b * _N : (b + 1) * _N]
            d = np.abs(sb[:, None] - sb[None, :])
            conf = np.maximum(_MARGIN - d, 0.0)
            np.fill_diagonal(conf, 0.0)
            push += conf.sum() / (_N * (_N - 1))
    return np.array([_PULL_W * pull, _PUSH_W * push], dtype=np.float32)


def _run(pred, inds, **spmd_kwargs):
    """Returns (full_output, BassKernelResults)."""
    from concourse.bass_utils import run_bass_kernel_spmd

    nc = _get_program()
    in_maps = _make_in_maps(pred, inds)
    res = run_bass_kernel_spmd(nc, in_maps, core_ids=list(range(_NCORES)), **spmd_kwargs)
    return _combine([r["out"] for r in res.results]), res


def kernel(pred, inds):
    out, _ = _run(pred, inds)
    return out
